# revision 6
# baseline (speedup 1.0000x reference)
"""HCAM sparse-attention Trainium2 kernel.

Math (per image i, caption t):
  z = flattened masked cosine sims [K*L=1800]; sparsemax(z) = relu(z - tau*),
  tau* root of  r(tau) = sum relu(z - tau) = 1;  output = sum z * relu(z - tau*).

Device algorithm per core (16 i's, all 128 t's):
  - big f32r matmul imgs_n^T @ caps_n -> fg tiles [t=128, (i,l,k)] in SBUF
  - Newton on tau (vectorized over (i,t)): r via ScalarE relu+bias+accum,
    counts via VectorE is_gt+add-accum, update via small vector ops.
    tau0 from a host-computed Gaussian-model init (function of lens only).
  - finish: attn = relu(z - tau) (DVE dual-op), q = sum attn^2 (ACT Square
    accum), out = q + tau (since sum attn = 1 at the root).

Sharding: data-parallel over i (16 per core); caps replicated.
"""
import sys
sys.path.insert(0, '/opt/trn_rl_repo')
from contextlib import ExitStack
import math
import numpy as np

import concourse.bass as bass
import concourse.tile as tile
import concourse.mybir as mybir
from concourse import bacc
from concourse.bass_utils import run_bass_kernel_spmd

F32 = mybir.dt.float32
F32R = mybir.dt.float32r
BF16 = mybir.dt.bfloat16
AF = mybir.ActivationFunctionType
ALU = mybir.AluOpType

NCORES = 8
BI, BT, K, L, D = 128, 128, 36, 50, 1024
IPC = BI // NCORES            # images per core = 16
KL = K * L                    # 1800
NHALF = 2
HI = IPC // NHALF             # 8 i per half
NCH = D // 128                # 8 contraction chunks
NW = IPC * K                  # 576 moving columns
HW_COLS = HI * K              # 288 per half
ITERS = 4                     # Newton iterations
EPS = 1e-6

_CACHE = {}


def _build_nc():
    if "nc" in _CACHE:
        return _CACHE["nc"]
    nc = bacc.Bacc("TRN2", target_bir_lowering=False, debug=False,
                   num_devices=NCORES)
    a_d = nc.dram_tensor("a_t", [D, NW], F32R, kind="ExternalInput").ap()
    b_d = nc.dram_tensor("b_t", [D, L * BT], F32R, kind="ExternalInput").ap()
    ntau_d = nc.dram_tensor("ntau0", [BT, IPC], F32, kind="ExternalInput").ap()
    out_d = nc.dram_tensor("out_o", [BT, IPC], F32, kind="ExternalOutput").ap()

    with tile.TileContext(nc) as tc, ExitStack() as ctx:
        _body(tc, ctx, a_d, b_d, ntau_d, out_d)
    nc.compile()
    _CACHE["nc"] = nc
    return nc


def _body(tc, ctx, a_d, b_d, ntau_d, out_d):
    nc = tc.nc
    big = ctx.enter_context(tc.tile_pool(name="big", bufs=1))
    bpool = ctx.enter_context(tc.tile_pool(name="bstream", bufs=3))
    pspool = ctx.enter_context(tc.tile_pool(name="ps", bufs=8, space="PSUM"))
    scr = ctx.enter_context(tc.tile_pool(name="scr", bufs=2))
    scrd = ctx.enter_context(tc.tile_pool(name="scrd", bufs=2))
    sm = ctx.enter_context(tc.tile_pool(name="small", bufs=1))
    smt = ctx.enter_context(tc.tile_pool(name="smtmp", bufs=2))

    # ---- persistent tiles ----
    a_sb = big.tile([128, NCH * NW], F32R, tag="a_sb")       # chunk j at cols j*NW
    z_h = [big.tile([128, HI * KL], F32, tag=f"z{h}", name=f"z{h}")
           for h in range(NHALF)]
    ntau = sm.tile([128, IPC], F32, tag="ntau")              # -tau per (t, i)
    tau = sm.tile([128, IPC], F32, tag="tau")
    rs = sm.tile([128, IPC], F32, tag="rs")
    cs = sm.tile([128, IPC], F32, tag="cs")
    qs = sm.tile([128, IPC], F32, tag="qs")
    outt = sm.tile([128, IPC], F32, tag="outt")

    # ---- input loads ----
    for j in range(NCH):
        nc.sync.dma_start(out=a_sb[:, j * NW:(j + 1) * NW],
                          in_=a_d[j * 128:(j + 1) * 128, :])
    nc.sync.dma_start(out=ntau[:], in_=ntau_d)
    nc.vector.tensor_scalar_mul(tau[:], ntau[:], -1.0)

    b_view = b_d.rearrange("(j p) n -> p j n", j=NCH)        # [128, 8, 6400]
    for h in range(NHALF):
        zh = z_h[h]
        # ---- matmul phase: 50 l-blocks for this half (B re-streamed) ----
        for l in range(L):
            b_l = bpool.tile([128, NCH * 128], F32R, tag="b_l")
            nc.sync.dma_start(
                out=b_l[:].rearrange("p (j c) -> p j c", j=NCH),
                in_=b_view[:, :, l * 128:(l + 1) * 128])
            ps = pspool.tile([128, HW_COLS], F32, tag="ps")
            for j in range(NCH):
                nc.tensor.matmul(
                    ps[:],
                    lhsT=b_l[:, j * 128:(j + 1) * 128],
                    rhs=a_sb[:, j * NW + h * HW_COLS:
                             j * NW + h * HW_COLS + HW_COLS],
                    start=(j == 0), stop=(j == NCH - 1))
            # drain into z layout [t, (i_local, l, k)], alternating engines
            dst = zh[:].rearrange("p (i n) -> p i n", i=HI)[:, :, l * K:(l + 1) * K]
            if (l + h) % 2 == 0:
                nc.scalar.copy(out=dst, in_=ps[:])
            else:
                nc.vector.tensor_copy(dst, ps[:])

        # ---- Newton iterations ----
        i0 = h * HI
        for it in range(ITERS):
            for i in range(HI):
                col = i0 + i
                zsl = zh[:, i * KL:(i + 1) * KL]
                so = scr.tile([128, KL], F32, tag="scr")
                nc.scalar.activation(out=so[:], in_=zsl, func=AF.Relu,
                                     bias=ntau[:, col:col + 1], scale=1.0,
                                     accum_out=rs[:, col:col + 1])
                do = scrd.tile([128, KL], F32, tag="scrd")
                nc.vector.tensor_scalar(out=do[:], in0=zsl,
                                        scalar1=tau[:, col:col + 1],
                                        scalar2=None,
                                        op0=ALU.is_gt, op1=ALU.add,
                                        accum_out=cs[:, col:col + 1])
            # tau += (r - 1) / c   (on [128, HI] column block)
            rsl = rs[:, i0:i0 + HI]
            csl = cs[:, i0:i0 + HI]
            rc = smt.tile([128, HI], F32, tag="rc")
            nc.vector.reciprocal(out=rc[:], in_=csl)
            rm = smt.tile([128, HI], F32, tag="rm")
            nc.vector.tensor_scalar_add(rm[:], rsl, -1.0)
            st = smt.tile([128, HI], F32, tag="st")
            nc.vector.tensor_tensor(out=st[:], in0=rm[:], in1=rc[:],
                                    op=ALU.mult)
            nc.vector.tensor_tensor(out=tau[:, i0:i0 + HI],
                                    in0=tau[:, i0:i0 + HI], in1=st[:],
                                    op=ALU.add)
            nc.vector.tensor_scalar_mul(ntau[:, i0:i0 + HI],
                                        tau[:, i0:i0 + HI], -1.0)

        # ---- finish: attn = relu(z-tau) on DVE; q = sum attn^2 on ACT ----
        for i in range(HI):
            col = i0 + i
            zsl = zh[:, i * KL:(i + 1) * KL]
            at = scrd.tile([128, KL], F32, tag="scrd")
            nc.vector.tensor_scalar(out=at[:], in0=zsl,
                                    scalar1=ntau[:, col:col + 1], scalar2=0.0,
                                    op0=ALU.add, op1=ALU.max)
            so = scr.tile([128, KL], F32, tag="scr")
            nc.scalar.activation(out=so[:], in_=at[:], func=AF.Square,
                                 bias=0.0, scale=1.0,
                                 accum_out=qs[:, col:col + 1])
        # out = q + tau  (sum attn = 1 at the root)
        nc.vector.tensor_tensor(out=outt[:, i0:i0 + HI],
                                in0=qs[:, i0:i0 + HI],
                                in1=tau[:, i0:i0 + HI], op=ALU.add)

    nc.sync.dma_start(out=out_d, in_=outt[:])


# ---------------- host side ----------------

def _phi(u):
    return np.exp(-0.5 * u * u) / math.sqrt(2.0 * math.pi)


def _sf(u):
    return 0.5 * np.vectorize(math.erfc)(u / math.sqrt(2.0))


def _model_tau(n_valid):
    """Gaussian-model sparsemax threshold: solve m*sigma*(phi(u)-u*sf(u)) = 1."""
    sigma = 1.0 / math.sqrt(D)
    us = np.linspace(0.0, 4.0, 2001)
    g = sigma * (_phi(us) - us * _sf(us))        # decreasing in u
    ms = np.unique(n_valid)
    lut = {}
    for m in ms:
        j = int(np.searchsorted(-m * g, -1.0))
        j = min(j, len(us) - 1)
        lut[float(m)] = sigma * us[j]
    out = np.empty(n_valid.shape, np.float32)
    for idx, m in np.ndenumerate(n_valid):
        out[idx] = lut[float(m)]
    return out


def kernel(img_cls, imgs, cap_cls, caps, img_lens, cap_lens):
    imgs = np.asarray(imgs, np.float32)
    caps = np.asarray(caps, np.float32)
    img_lens = np.asarray(img_lens)
    cap_lens = np.asarray(cap_lens)

    # normalize (+eps) then zero masked rows — mirrors reference, which
    # masks fg entries to 0 after the cosine-sim einsum
    im = imgs + np.float32(EPS)
    cp = caps + np.float32(EPS)
    im = im / np.linalg.norm(im.astype(np.float64), axis=-1,
                             keepdims=True).astype(np.float32)
    cp = cp / np.linalg.norm(cp.astype(np.float64), axis=-1,
                             keepdims=True).astype(np.float32)
    im = im * (np.arange(K)[None, :] < img_lens[:, None])[:, :, None]
    cp = cp * (np.arange(L)[None, :] < cap_lens[:, None])[:, :, None]

    # b_t[d, l*128 + t] = cp[t, l, d]  (shared across cores)
    b_t = np.ascontiguousarray(cp.transpose(2, 1, 0).reshape(D, L * BT))

    # tau model init per (t, i) for each core; negative, with shrink margin
    n_valid = (img_lens[:, None] * cap_lens[None, :]).astype(np.float32)
    tau0 = _model_tau(n_valid)                       # [bi, bt]

    nc = _build_nc()
    in_maps = []
    for c in range(NCORES):
        sl = slice(c * IPC, (c + 1) * IPC)
        a_t = np.ascontiguousarray(
            im[sl].transpose(2, 0, 1).reshape(D, NW))       # [d, i*K+k]
        ntau0 = np.ascontiguousarray(-tau0[sl].T)           # [t, i_local]
        in_maps.append({"a_t": a_t, "b_t": b_t,
                        "ntau0": ntau0.astype(np.float32)})
    res = run_bass_kernel_spmd(nc, in_maps, list(range(NCORES)))

    out = np.empty((BI, BT), np.float32)
    for c in range(NCORES):
        out[c * IPC:(c + 1) * IPC, :] = res.results[c]["out_o"].T
    return out
